# revision 29
# baseline (speedup 1.0000x reference)
"""CrossBandAttention Trainium2 kernel.

Math: 3 bands [B,C,H,W] -> per-band 1x1 conv (C->E) -> MHA over the 3-band
sequence per pixel -> out-proj -> per-band 1x1 conv (E->C) -> gated residual.

All linear stages are merged on the host into tiny per-band matrices acting on
the raw 9 input channels (3 bands x 3 chans), so the device kernel is, per
pixel: a handful of [9 -> 72] matmuls, the 3x3x8 score bilinear form, exp,
softmax-normalize, attn-weighted sum, residual. Layout is feature-major:
SBUF/PSUM tiles [rows, FD-pixels], pixels streamed in FD=512 chunks.

Row orderings:
  aug rows   r = 3j + c            (band j, channel c)       [9]
  t/y rows   m = i*24 + h*3 + a    (query band i, head h, a)  [72]
  e/z/T rows m = c*24 + i*8 + h    (payload chan c, i, h)     [72]
  den rows   m = c*24 + i*8 + h    (c-replicated)             [72]
  out rows   m = 3i + c                                        [9]
"""

import math

import numpy as np

B, C, H, W = 4, 3, 256, 256
E, HEADS, HD = 64, 8, 8
NCORES = 8
PIX = B * H * W // NCORES  # 32768 pixels per core
FD = 512                   # pixels per chunk
NCHUNK = PIX // FD

IH = 24   # (i, h) pairs
ROWS = 72


def _merged_weights(Wp, bp, in_proj_w, in_proj_b, out_proj_w, out_proj_b,
                    Wo, bo, gates):
    """Fold every linear stage into small fp32 matrices. float64 internally."""
    f8 = np.float64
    Wp, bp = Wp.astype(f8), bp.astype(f8)
    ipw, ipb = in_proj_w.astype(f8), in_proj_b.astype(f8)
    opw, opb = out_proj_w.astype(f8), out_proj_b.astype(f8)
    Wo, bo = Wo.astype(f8), bo.astype(f8)
    g = gates.astype(f8)
    w = np.exp(g - g.max())
    w /= w.sum()

    # per-band merged q/k/v from (3 chans + const): [3][64, 4]
    QA = np.zeros((3, E, 4))
    KA = np.zeros((3, E, 4))
    VA = np.zeros((3, E, 4))
    for j in range(3):
        for blk, M in ((0, QA), (1, KA), (2, VA)):
            r0 = blk * E
            M[j, :, :3] = ipw[r0:r0 + E] @ Wp[j]
            M[j, :, 3] = ipw[r0:r0 + E] @ bp[j] + ipb[r0:r0 + E]

    # score bilinear forms: S[i,j,h] in R^{4x4}
    S = np.zeros((3, 3, HEADS, 4, 4))
    for i in range(3):
        for j in range(3):
            for h in range(HEADS):
                qb = QA[i, h * HD:(h + 1) * HD]      # [8, 4]
                kb = KA[j, h * HD:(h + 1) * HD]
                S[i, j, h] = qb.T @ kb / math.sqrt(HD)

    # output-side merge: M_ih [3, 8] maps head-h v-components to band-i chans
    WoP = np.einsum('ice,ef->icf', Wo, opw)          # [3, C, E]
    Mih = np.zeros((3, HEADS, 3, HD))
    for i in range(3):
        for h in range(HEADS):
            Mih[i, h] = w[i] * WoP[i][:, h * HD:(h + 1) * HD]
    b_eff = (np.einsum('ice,e->ic', Wo, opb) + bo) * w[:, None]   # [3, C]

    def em(c, i, h):  # e/z row index
        return c * 24 + i * 8 + h

    # ---- pair-product form for the score bilinears ----
    # m2[p*9 + a*3 + b] = band_{P1(p)}[a] * band_{P2(p)}[b] over 6 band pairs
    PAIRS = [(0, 1), (0, 2), (1, 2), (0, 0), (1, 1), (2, 2)]
    M2 = 54
    RA = np.zeros((9, M2))
    RB = np.zeros((9, M2))
    for p, (p1, p2) in enumerate(PAIRS):
        for a in range(3):
            for b in range(3):
                RA[3 * p1 + a, p * 9 + a * 3 + b] = 1.0
                RB[3 * p2 + b, p * 9 + a * 3 + b] = 1.0

    WS = np.zeros((3, M2, ROWS))           # m2 rows -> e rows, per source band j
    for j in range(3):
        for c in range(3):
            for i in range(3):
                for h in range(HEADS):
                    m = em(c, i, h)
                    if i == j:
                        p = PAIRS.index((i, i))
                        for a in range(3):
                            for b in range(3):
                                WS[j, p * 9 + a * 3 + b, m] += S[i, j, h][a, b]
                    else:
                        p = PAIRS.index((min(i, j), max(i, j)))
                        p1, p2 = PAIRS[p]
                        for a in range(3):
                            for b in range(3):
                                if (p1, p2) == (i, j):
                                    WS[j, p * 9 + a * 3 + b, m] += S[i, j, h][a, b]
                                else:
                                    WS[j, p * 9 + a * 3 + b, m] += S[i, j, h][b, a]

    VAl = np.stack([VA[j][:, :3] for j in range(3)])       # [3, 64, 3]
    vc = np.stack([VA[j][:, 3] for j in range(3)])         # [3, 64]
    Wz = np.zeros((3, 9, ROWS))
    zc = np.zeros((3, ROWS))
    for j in range(3):
        for c in range(3):
            for i in range(3):
                for h in range(HEADS):
                    m = em(c, i, h)
                    for b in range(3):
                        Wz[j, 3 * j + b, m] = Mih[i, h][c] @ VAl[j, h * HD:(h + 1) * HD, b]
                    zc[j, m] = Mih[i, h][c] @ vc[j, h * HD:(h + 1) * HD]

    Lin = np.zeros((3, 9, ROWS))
    EB = np.zeros((ROWS, 3))
    for j in range(3):
        for c in range(3):
            for i in range(3):
                for h in range(HEADS):
                    m = em(c, i, h)
                    for a in range(3):
                        Lin[j, 3 * i + a, m] += S[i, j, h][a, 3]
                    for b in range(3):
                        Lin[j, 3 * j + b, m] += S[i, j, h][3, b]
                    EB[m, j] = S[i, j, h][3, 3]

    Ires = np.eye(9)

    # ---- W72 blocks ----
    I72 = np.eye(ROWS)
    Wh = np.zeros((ROWS, 9))
    for c in range(3):
        for i in range(3):
            for h in range(HEADS):
                Wh[em(c, i, h), 3 * i + c] = 1.0

    Ibc = np.zeros((IH, ROWS))             # den c-replication [24 -> 72]
    for c in range(3):
        for i in range(3):
            for h in range(HEADS):
                Ibc[i * 8 + h, em(c, i, h)] = 1.0

    f4 = np.float32
    # WK9: all K=9 weights, stored in rows 64-72 of a [73, *] tile so their
    # matmuls can read the aug rows of the combined [m2; gap; aug] rhs tile
    # (rhs/lhsT base partition must be 32-aligned -> 64).
    WK9 = np.concatenate([RA, RB] + [Wz[j] for j in range(3)] + [Ires], axis=1)
    WK9_full = np.zeros((73, WK9.shape[1]))
    WK9_full[64:73] = WK9
    # W63: per-band score matmul over the combined rhs [m2(0:54); 0(54:64);
    # aug(64:73)] — rows 0-53 = WS_j, rows 64-72 = Lin_j.
    W63 = np.zeros((73, 3 * ROWS))
    for j in range(3):
        W63[0:54, j * ROWS:(j + 1) * ROWS] = WS[j]
        W63[64:73, j * ROWS:(j + 1) * ROWS] = Lin[j]
    return {
        'WK9': WK9_full.astype(f4),        # [73, 333]
        'W63': W63.astype(f4),             # [73, 216]
        'WH': Wh.astype(f4),               # [72, 9]
        'EB': EB.astype(f4),               # [72, 3]
        'ZC': zc.T.astype(f4),             # [72, 3]
        'BIAS9': b_eff.reshape(9, 1).astype(f4),
    }


# column offsets inside WK9 / W63
M2 = 54
AUGR = 64          # aug rows live at [64:73] of the combined rhs tile
K9_RA, K9_RB = 0, M2
K9_Z = [2 * M2 + j * ROWS for j in range(3)]
K9_IRES = 2 * M2 + 3 * ROWS
W63_WS = [j * ROWS for j in range(3)]


def golden_core(xb, wts):
    """Numpy emulation of the device program for one core. xb: [3][3, pix]."""
    pix = xb[0].shape[1]
    aug = np.concatenate([xb[0], xb[1], xb[2]], axis=0).astype(np.float32)  # [9, pix]
    WK9, W63 = wts['WK9'], wts['W63']
    comb = np.zeros((73, pix), np.float32)
    comb[AUGR:AUGR + 9] = aug
    ra = WK9[AUGR:, K9_RA:K9_RA + M2].T @ aug
    rb = WK9[AUGR:, K9_RB:K9_RB + M2].T @ aug
    comb[0:M2] = ra * rb
    es, ps = [], []
    # den replicated to 72 rows so every elementwise op stays lane-aligned
    ibc = np.zeros((IH, ROWS), np.float32)
    for c in range(3):
        for k in range(IH):
            ibc[k, c * IH + k] = 1.0
    for j in range(3):
        sc = W63[:, W63_WS[j]:W63_WS[j] + ROWS].T @ comb
        e = np.exp(sc + wts['EB'][:, j:j + 1])
        es.append(e)
    for j in range(3):
        z = WK9[AUGR:, K9_Z[j]:K9_Z[j] + ROWS].T @ aug
        ps.append((z + wts['ZC'][:, j:j + 1]) * es[j])
    T = ps[0] + ps[1] + ps[2]
    den = es[0][:IH] + es[1][:IH] + es[2][:IH]
    rcp = np.zeros((ROWS, pix), np.float32)
    rcp[:IH] = 1.0 / den
    msb = T.copy()
    for c in range(3):
        msb[c * IH:(c + 1) * IH] *= rcp[:IH]
    out9 = wts['WH'].T @ msb \
        + WK9[AUGR:, K9_IRES:K9_IRES + 9].T @ aug + wts['BIAS9']
    return out9.astype(np.float32)  # [9, pix] rows 3i+c


def build_program(pix=PIX, fd=FD, reps=1):
    """reps>1 wraps the whole pixel loop in a device-side For_i so kernel
    time can be measured as the wall-clock delta between rep counts."""
    import concourse.bacc as bacc
    import concourse.mybir as mybir
    from concourse.tile import TileContext
    from contextlib import nullcontext

    f32 = mybir.dt.float32
    AF = mybir.ActivationFunctionType
    nc = bacc.Bacc("TRN2")

    xall = nc.dram_tensor("xall", [9, pix], f32, kind="ExternalInput").ap()
    wk9_d = nc.dram_tensor("wk9", [73, 333], f32, kind="ExternalInput").ap()
    w63_d = nc.dram_tensor("w63", [73, 216], f32, kind="ExternalInput").ap()
    wh_d = nc.dram_tensor("wh", [ROWS, 9], f32, kind="ExternalInput").ap()
    eb_d = nc.dram_tensor("eb", [ROWS, 3], f32, kind="ExternalInput").ap()
    zc_d = nc.dram_tensor("zc", [ROWS, 3], f32, kind="ExternalInput").ap()
    b9_d = nc.dram_tensor("b9", [9, 1], f32, kind="ExternalInput").ap()
    oall = nc.dram_tensor("oall", [9, pix], f32, kind="ExternalOutput").ap()

    with TileContext(nc) as tc:
        with (
            tc.tile_pool(name="const", bufs=1) as cp,
            tc.tile_pool(name="sb", bufs=3) as sb,
            tc.tile_pool(name="ps", bufs=2, space="PSUM") as pp,
        ):
            wk9 = cp.tile([73, 333], f32)
            w63 = cp.tile([73, 216], f32)
            wh = cp.tile([ROWS, 9], f32)
            eb = cp.tile([ROWS, 3], f32)
            zc = cp.tile([ROWS, 3], f32)
            b9 = cp.tile([9, 1], f32)
            nc.sync.dma_start(wk9[:], wk9_d)
            nc.sync.dma_start(w63[:], w63_d)
            nc.sync.dma_start(wh[:], wh_d)
            nc.sync.dma_start(eb[:], eb_d)
            nc.sync.dma_start(zc[:], zc_d)
            nc.sync.dma_start(b9[:], b9_d)
            # Dummy PE consumers of each const so later matmuls never wait on
            # two DMA-HW queues at once (the LDWEIGHTS struct can hold only
            # one DGE sync-wait).
            warm = pp.tile([1, 8], f32, tag="A")
            for wt in (wk9, w63, wh):
                nc.tensor.matmul(warm[:], wt[0:9, 0:1], wt[0:9, 0:8],
                                 start=True, stop=True)

            rep_ctx = tc.For_i(0, reps, 1) if reps > 1 else nullcontext()
            with rep_ctx:
              for ci in range(pix // fd):
                o = ci * fd
                # combined rhs: rows 0-53 = m2 (DVE), 54-63 = zeros, 64-72 = aug
                comb = sb.tile([73, fd], f32, tag="comb")
                nc.gpsimd.memset(comb[32:AUGR, :], 0.0)  # covers the 54-63 gap
                nc.sync.dma_start(comb[AUGR:AUGR + 9, :], xall[:, o:o + fd])
                augr = comb[AUGR:AUGR + 9, :]

                ra_p = pp.tile([M2, fd], f32, tag="A")
                nc.tensor.matmul(ra_p[:], wk9[AUGR:, K9_RA:K9_RA + M2], augr,
                                 start=True, stop=True)
                rb_p = pp.tile([M2, fd], f32, tag="y")
                nc.tensor.matmul(rb_p[:], wk9[AUGR:, K9_RB:K9_RB + M2], augr,
                                 start=True, stop=True)
                rb_s = sb.tile([M2, fd], f32, tag="rb")
                nc.scalar.copy(rb_s[:], rb_p[:])
                nc.vector.tensor_mul(comb[0:M2, :], ra_p[:], rb_s[:])

                es = []
                for j in range(3):
                    sc_p = pp.tile([ROWS, fd], f32, tag="sc")
                    nc.tensor.matmul(sc_p[:], w63[:, W63_WS[j]:W63_WS[j] + ROWS],
                                     comb[:], start=True, stop=True)
                    e_j = sb.tile([ROWS, fd], f32, tag=f"e{j}")
                    nc.scalar.activation(e_j[:], sc_p[:], AF.Exp,
                                         bias=eb[:, j:j + 1])
                    es.append(e_j)

                ps_l = []
                for j in range(3):
                    z_p = pp.tile([ROWS, fd], f32, tag="z")
                    nc.tensor.matmul(z_p[:], wk9[AUGR:, K9_Z[j]:K9_Z[j] + ROWS],
                                     augr, start=True, stop=True)
                    p_j = sb.tile([ROWS, fd], f32, tag=f"p{j}")
                    # p = (z + zc_j) * e   (zc_j is a per-partition scalar)
                    nc.vector.scalar_tensor_tensor(
                        p_j[:], z_p[:], zc[:, j:j + 1], es[j][:],
                        op0=mybir.AluOpType.add, op1=mybir.AluOpType.mult)
                    ps_l.append(p_j)

                t01 = sb.tile([ROWS, fd], f32, tag="t01")
                nc.vector.tensor_add(t01[:], ps_l[0][:], ps_l[1][:])
                tt = sb.tile([ROWS, fd], f32, tag="tt")
                nc.vector.tensor_add(tt[:], t01[:], ps_l[2][:])

                d01 = sb.tile([IH, fd], f32, tag="d01")
                nc.vector.tensor_add(d01[:], es[0][0:IH, :], es[1][0:IH, :])
                rcp = sb.tile([ROWS, fd], f32, tag="rcp")
                nc.vector.tensor_add(rcp[0:IH, :], d01[:], es[2][0:IH, :])
                nc.vector.reciprocal(rcp[0:IH, :], rcp[0:IH, :])
                # replicate 1/den to all three c-blocks (cross-partition: DMA)
                nc.sync.dma_start(rcp[IH:2 * IH, :], rcp[0:IH, :])
                nc.sync.dma_start(rcp[2 * IH:3 * IH, :], rcp[0:IH, :])
                msb = sb.tile([ROWS, fd], f32, tag="msb")
                nc.vector.tensor_mul(msb[:], tt[:], rcp[:])

                out9_p = pp.tile([9, fd], f32, tag="y")
                nc.tensor.matmul(out9_p[:], wh[:], msb[:],
                                 start=True, stop=False)
                # residual band add; gate bias added in the Identity copy
                nc.tensor.matmul(out9_p[:], wk9[AUGR:, K9_IRES:K9_IRES + 9],
                                 augr, start=False, stop=True)
                res = sb.tile([9, fd], f32, tag="res")
                nc.scalar.activation(res[:], out9_p[:], AF.Identity,
                                     bias=b9[:, 0:1])
                nc.sync.dma_start(oall[:, o:o + fd], res[:])
    nc.compile()
    return nc


def _shard_inputs(band0, band1, band2, wts):
    bands = [np.ascontiguousarray(b, dtype=np.float32)
             for b in (band0, band1, band2)]
    in_maps = []
    for k in range(NCORES):
        b, half = k // 2, k % 2
        m = dict(wk9=wts['WK9'], w63=wts['W63'], wh=wts['WH'],
                 eb=wts['EB'], zc=wts['ZC'], b9=wts['BIAS9'])
        m["xall"] = np.concatenate(
            [bands[j][b, :, half * 128:half * 128 + 128, :].reshape(3, PIX)
             for j in range(3)], axis=0)
        in_maps.append(m)
    return in_maps


def _unshard(results):
    full = [np.empty((B, C, H, W), np.float32) for _ in range(3)]
    for k in range(NCORES):
        b, half = k // 2, k % 2
        for j in range(3):
            full[j][b, :, half * 128:half * 128 + 128, :] = \
                results[k]["oall"][3 * j:3 * j + 3].reshape(3, 128, W)
    return tuple(full)


def kernel(band0, band1, band2, Wp, bp, in_proj_w, in_proj_b,
           out_proj_w, out_proj_b, Wo, bo, gates):
    from concourse.bass_utils import run_bass_kernel_spmd

    wts = _merged_weights(np.asarray(Wp), np.asarray(bp), np.asarray(in_proj_w),
                          np.asarray(in_proj_b), np.asarray(out_proj_w),
                          np.asarray(out_proj_b), np.asarray(Wo),
                          np.asarray(bo), np.asarray(gates))
    nc = build_program()
    in_maps = _shard_inputs(np.asarray(band0), np.asarray(band1),
                            np.asarray(band2), wts)
    r = run_bass_kernel_spmd(nc, in_maps, core_ids=list(range(NCORES)))
    return _unshard(r.results)
